# revision 9
# baseline (speedup 1.0000x reference)
"""Trainium2 Bass kernel for nn_ConvTemporalGraphical1 (gnn_message_passing).

Reference computation (N=64, CIN=192, T=256, V=25, K=3, COUT=64, S=5):
    y   = einsum('nctv,oc->notv', x, Wc) + bc          # 1x1 conv, o = K*COUT
    out = einsum('nkctv,kvw->nctw', y.reshape(N,K,COUT,T,V), A)
    graphs = f(mean_T(y), node_type, W1, b1, W2, b2)   # tiny, linear in mean_T(y)

Strategy (pure data parallel over N, 8 cores x 8 batch elements):
  - Per batch element n, per chunk of 4 timesteps (100 positions = 4t x 25v):
      yT[pos, o] = x_chunk.T @ Wc.T        (PE, contraction over CIN in 2 pieces)
      out[c, (t,w)] += yT[:, k-slice].T @ BD_k   for k=0..2 (PSUM-accumulated)
    where BD_k is the host-built block-diagonal replication of A[k] over the 4
    timesteps. This yields `out` directly in its HBM layout - no transposes.
  - A third matmul per chunk against a host-built selector E accumulates
    ysum[v, o] = sum_t yT[(t,v), o] in PSUM across all 256 timesteps; everything
    downstream of the T-mean (graphs) is linear and tiny, finished on host.
  - bc is folded in on host: into the `graphs` epilogue, and into a (c, t*w)
    bias map added during the PSUM->SBUF eviction of `out`.
"""

import sys

sys.path.insert(0, "/opt/trn_rl_repo")

import numpy as np

N, CIN, T, V = 64, 192, 256, 25
K, COUT, S = 3, 64, 5
O = K * COUT  # 192

NCORES = 8
NPC = N // NCORES  # 8 batch elements per core
TCH = 4            # timesteps per compute chunk
PCH = TCH * V      # 100 positions per chunk
QT = 64            # timesteps per DMA tile
QPOS = QT * V      # 1600
NQ = T // QT       # 4 DMA tiles per batch element
CPQ = QT // TCH    # 16 chunks per DMA tile


def _split_multi_waits(nc):
    """The walrus build in this container accepts at most ONE sync wait per
    engine instruction (setupSyncWait: 'Too many sync wait commands').  Tile
    freely emits 2-3 waits per instruction, so post-process the scheduled BIR:
    hoist all but one wait onto EventSemaphore instructions (the same carrier
    the framework's own barriers use) inserted just before, on the same engine.
    Drains are left alone (the standard kernel tail compiles with 11 waits)."""
    import json

    from concourse import mybir

    data = json.loads(mybir.module_to_json_bytes(nc.m))
    nsplit = 0
    for fn in data["functions"]:
        for blk in fn["blocks"]:
            out = []
            for inst in blk["instructions"]:
                si = inst.get("sync_info")
                if si:
                    waits = si.get("on_wait") or []
                    if len(waits) > 1:
                        for w in waits[:-1]:
                            nsplit += 1
                            # the simulator requires >=1 update on an
                            # EventSemaphore; a +0 add is a hardware no-op
                            upd = {
                                "ant_name": w["ant_name"],
                                "id": w["id"],
                                "sync_type": "semaphore",
                                "update_mode": "sem-add-imm",
                                "update_value": 0,
                            }
                            out.append(
                                {
                                    "debug": inst.get("debug", 0),
                                    "engine": inst["engine"],
                                    "ins": [],
                                    "outs": [],
                                    "name": f"evw-{nsplit}",
                                    "opcode": "EventSemaphore",
                                    "sync_info": {"on_update": [upd], "on_wait": [w]},
                                }
                            )
                        si["on_wait"] = [waits[-1]]
                out.append(inst)
            blk["instructions"] = out
    nc.m = mybir.module_from_json_bytes(json.dumps(data).encode())
    return nsplit


def _build_nc(patch=True):
    from concourse import bass, mybir, tile
    from contextlib import ExitStack

    f32 = mybir.dt.float32
    nc = bass.Bass()

    x_d = nc.declare_dram_parameter("x", [NPC, CIN, T * V], f32, False)
    wa_d = nc.declare_dram_parameter("wa", [128, O], f32, False)
    wb_d = nc.declare_dram_parameter("wb", [CIN - 128, O], f32, False)
    bd_d = nc.declare_dram_parameter("bd", [PCH, K * PCH], f32, False)
    e_d = nc.declare_dram_parameter("e", [PCH, V], f32, False)
    bias_d = nc.declare_dram_parameter("bias", [COUT, PCH], f32, False)
    out_d = nc.declare_dram_parameter("out", [NPC, COUT, T * V], f32, True)
    ysum_d = nc.declare_dram_parameter("ysum", [NPC, V, O], f32, True)

    with tile.TileContext(nc) as tc, ExitStack() as ctx:
        constp = ctx.enter_context(tc.tile_pool(name="const", bufs=1))
        xp = ctx.enter_context(tc.tile_pool(name="xin", bufs=2))
        ytp = ctx.enter_context(tc.tile_pool(name="yt", bufs=3))
        stp = ctx.enter_context(tc.tile_pool(name="stage", bufs=2))
        psy = ctx.enter_context(
            tc.tile_pool(name="ps_y", bufs=2, space=bass.MemorySpace.PSUM)
        )
        pso = ctx.enter_context(
            tc.tile_pool(name="ps_o", bufs=2, space=bass.MemorySpace.PSUM)
        )
        pss = ctx.enter_context(
            tc.tile_pool(name="ps_s", bufs=2, space=bass.MemorySpace.PSUM)
        )

        wa = constp.tile([128, O], f32)
        nc.sync.dma_start(out=wa[:], in_=wa_d[:])
        wb = constp.tile([CIN - 128, O], f32)
        nc.sync.dma_start(out=wb[:], in_=wb_d[:])
        bd = constp.tile([PCH, K * PCH], f32)
        nc.sync.dma_start(out=bd[:], in_=bd_d[:])
        e = constp.tile([PCH, V], f32)
        nc.sync.dma_start(out=e[:], in_=e_d[:])
        bias = constp.tile([COUT, PCH], f32)
        nc.sync.dma_start(out=bias[:], in_=bias_d[:])

        for n in range(NPC):
            ysum_ps = pss.tile([V, O], f32)
            for q in range(NQ):
                xa = xp.tile([128, QPOS], f32, tag="xa")
                nc.sync.dma_start(
                    out=xa[:], in_=x_d[n, 0:128, q * QPOS : (q + 1) * QPOS]
                )
                xb = xp.tile([CIN - 128, QPOS], f32, tag="xb")
                nc.sync.dma_start(
                    out=xb[:], in_=x_d[n, 128:CIN, q * QPOS : (q + 1) * QPOS]
                )
                stage = stp.tile([COUT, QPOS], f32, tag="stage")
                for j in range(CPQ):
                    sl = slice(j * PCH, (j + 1) * PCH)
                    y_ps = psy.tile([PCH, O], f32, tag="y_ps")
                    nc.tensor.matmul(y_ps[:], xa[:, sl], wa[:], start=True, stop=False)
                    nc.tensor.matmul(y_ps[:], xb[:, sl], wb[:], start=False, stop=True)
                    yt = ytp.tile([PCH, O], f32, tag="yt")
                    nc.scalar.copy(yt[:], y_ps[:])
                    first = q == 0 and j == 0
                    last = q == NQ - 1 and j == CPQ - 1
                    nc.tensor.matmul(
                        ysum_ps[:], e[:], yt[:],
                        start=first, stop=last, skip_group_check=True,
                    )
                    o_ps = pso.tile([COUT, PCH], f32, tag="o_ps")
                    for k in range(K):
                        nc.tensor.matmul(
                            o_ps[:],
                            yt[:, k * COUT : (k + 1) * COUT],
                            bd[:, k * PCH : (k + 1) * PCH],
                            start=(k == 0), stop=(k == K - 1),
                            skip_group_check=True,
                        )
                    nc.vector.tensor_add(stage[:, sl], o_ps[:], bias[:])
                nc.sync.dma_start(
                    out=out_d[n, :, q * QPOS : (q + 1) * QPOS], in_=stage[:]
                )
            ys = stp.tile([V, O], f32, tag="ys")
            nc.scalar.copy(ys[:], ysum_ps[:])
            nc.sync.dma_start(out=ysum_d[n], in_=ys[:])

    if patch:
        _split_multi_waits(nc)
    return nc


def _host_consts(A, Wc, bc):
    A = np.asarray(A, np.float32)
    Wc = np.asarray(Wc, np.float32)
    bc = np.asarray(bc, np.float32)
    WcT = np.ascontiguousarray(Wc.T)  # (CIN, O)
    wa = WcT[:128].copy()
    wb = WcT[128:].copy()
    bd = np.zeros((PCH, K * PCH), np.float32)
    for k in range(K):
        for t in range(TCH):
            bd[t * V : (t + 1) * V, k * PCH + t * V : k * PCH + (t + 1) * V] = A[k]
    e = np.tile(np.eye(V, dtype=np.float32), (TCH, 1))
    Asum = A.sum(axis=1)  # (K, V)
    bias_cw = np.einsum("kc,kw->cw", bc.reshape(K, COUT), Asum).astype(np.float32)
    bias = np.tile(bias_cw, (1, TCH))  # (COUT, PCH)
    return wa, wb, bd, e, bias


def _epilogue(ysum_all, A, node_type, bc, W1, b1, W2, b2):
    """ysum_all: (N, V, O) = sum_t (Wc@x)^T per batch elem. Returns graphs."""
    bc = np.asarray(bc, np.float32)
    W1 = np.asarray(W1, np.float32)
    b1 = np.asarray(b1, np.float32)
    W2 = np.asarray(W2, np.float32)
    b2 = np.asarray(b2, np.float32)
    nt = np.asarray(node_type).astype(np.int64)

    ybar = ysum_all.transpose(0, 2, 1) / np.float32(T) + bc[None, :, None]  # (N,O,V)
    x1m = np.einsum("po,nov->npv", W1, ybar) + b1[None, :, None]  # (N, S*COUT, V)
    x2m = np.einsum("po,nov->npv", W2, ybar) + b2[None, :, None]
    x1m = x1m.reshape(N, S, COUT, V)
    x2m = x2m.reshape(N, S, COUT, V)
    masks = (nt[None, :] == np.arange(S)[:, None]).astype(np.float32)  # (S,V)
    counts = masks.sum(axis=-1)  # (S,)
    sem = np.einsum("nscv,sv->nsc", x1m, masks) / counts[None, :, None]
    graphs = sem[:, :, :, None, None, None] - x2m[:, :, :, None, None, :]
    return graphs.astype(np.float32)


def _prepare(inputs):
    x = np.ascontiguousarray(np.asarray(inputs["x"], np.float32)).reshape(
        NCORES, NPC, CIN, T * V
    )
    wa, wb, bd, e, bias = _host_consts(inputs["A"], inputs["Wc"], inputs["bc"])

    nc = _build_nc()
    in_maps = [
        {"x": x[c], "wa": wa, "wb": wb, "bd": bd, "e": e, "bias": bias}
        for c in range(NCORES)
    ]
    return nc, in_maps


def _run(inputs, trace=False):
    from concourse.bass_utils import run_bass_kernel_spmd

    nc, in_maps = _prepare(inputs)
    res = run_bass_kernel_spmd(nc, in_maps, list(range(NCORES)), trace=trace)

    out = np.concatenate([res.results[c]["out"] for c in range(NCORES)], axis=0)
    out = out.reshape(N, COUT, T, V)
    ysum = np.concatenate([res.results[c]["ysum"] for c in range(NCORES)], axis=0)

    graphs = _epilogue(
        ysum, inputs["A"], inputs["node_type"], inputs["bc"],
        inputs["W1"], inputs["b1"], inputs["W2"], inputs["b2"],
    )
    A_out = np.asarray(inputs["A"], np.float32)
    return (out, A_out, graphs), res


def kernel(**inputs):
    outputs, _ = _run(inputs, trace=False)
    return outputs


def kernel_timed(**inputs):
    outputs, res = _run(inputs, trace=False)
    return outputs, res
